# revision 14
# baseline (speedup 1.0000x reference)
"""Trainium2 Bass kernel for a single causal attention head.

Problem: x:(8,2048,1024) f32, per-head projections wq/wk/wv:(64,1024),
biases (64,). Output: softmax(causal(q k^T / sqrt(64))) @ v : (8,2048,64).

Strategy:
  - Data-parallel: batch b -> core b (8 cores, 1 batch each).
  - Host prep packs every input into partition-major, fully contiguous
    per-partition lines so each DMA is ~128 large descriptors:
      * xp:(P, NCH*DT*CH) fp16 - x[b] chunk-major/d-major per partition
        (8KB contiguous per partition per chunk).
      * wall:(P, DT*(P+HD)) fp16 - [wq*s|wk] and wv interleaved per d-tile.
      * bb:(P, 2) f32 - [bq*s;bk] and [bv;bv] columns.
  - Device (per core):
      * qk1 = [wq|wk]^T.T @ x: rows 0-63 = Q^T, rows 64-127 = K^T (PSUM
        accumulate over 8 d-tiles, fp16 matmuls, N=512 chunks).
      * qk2 = half-swapped copy of qk1 -> both Q^T and K^T live on both
        partition halves; scores for two k-tiles share the PE array via
        row packing.
      * vT (64,T) fp16, transposed back to (T,64) tiles via fp16 PE
        transpose, augmented with a ones column (softmax denominator
        rides along the PV matmul).
      * S^T = K^T.T @ Q^T per k-tile; P^T = exp(S^T) on ACT; causal mask
        via gpsimd affine_select restricted to the 128-col diagonal band.
      * Diagonal pairs run FIRST per chunk with column-trimmed scores/
        exp/mask/PV (fully-masked columns never computed); non-diagonal
        pairs follow full-range.
      * O^T_aug[65, T] accumulated in PSUM over k-tiles; row 64 = sum_j P^T.
      * attention for chunk ci emitted right after projection chunk ci.
  - Host post: out[b] = (O^T[0:64] / O^T[64:65]).T  (softmax normalization).
"""

import numpy as np

B, T, D, HD = 8, 2048, 1024, 64
P = 128          # SBUF partitions
CH = 512         # q-chunk (matmul moving dim)
NCH = T // CH    # 4
DT = D // P      # 8 d-tiles
NKT = T // P     # 16 k-tiles
NWARM = 9        # PE clock-ramp warmup matmuls
DHA = 5          # d-tiles in the first x half-load (two DMA queues)

LAST_RESULTS = None      # BassKernelResults of the most recent run (for test.py)


def _build_module(legalize=True):
    import concourse.bass as bass
    import concourse.mybir as mybir
    from concourse.tile import TileContext

    from concourse.masks import make_identity
    F32 = mybir.dt.float32
    F16 = mybir.dt.float16

    nc = bass.Bass("TRN2", target_bir_lowering=True)

    xp = nc.dram_tensor("xp", (P, NCH * DT * CH), F16, kind="ExternalInput")
    w1d = nc.dram_tensor("w1d", (P, DT * P), F16, kind="ExternalInput")
    wvd = nc.dram_tensor("wvd", (P, DT * HD), F16, kind="ExternalInput")
    bb = nc.dram_tensor("bb", (P, 2), F32, kind="ExternalInput")
    outT = nc.dram_tensor("outT", (HD + 1, T), F32, kind="ExternalOutput")

    with TileContext(nc) as tc:
        with (
            tc.tile_pool(name="const", bufs=1) as const,
            tc.tile_pool(name="acts", bufs=1) as acts,
            tc.tile_pool(name="proj_ps", bufs=2, space="PSUM") as proj_ps,
            tc.tile_pool(name="tr_ps", bufs=1, space="PSUM") as tr_ps,
            tc.tile_pool(name="s_ps", bufs=2, space="PSUM") as s_ps,
            tc.tile_pool(name="o_ps", bufs=1, space="PSUM") as o_ps,
            tc.tile_pool(name="pwork", bufs=6) as pwork,
            tc.tile_pool(name="owork", bufs=2) as owork,
        ):
            # ---- PE warm-up first: throwaway matmuls keep the PE busy
            # through its clock-ramp window so real matmuls run at full
            # speed. Gated only on the wscr memset, not on any DMA. Any PE
            # idle gap resets the clock ramp, so the warmup count is sized
            # to bridge until the first x half lands.
            wscr = const.tile([P, CH], F16, name="wscr")
            nc.vector.memset(wscr[:], 0.0)
            for wu in range(NWARM):
                pswu = proj_ps.tile([P, CH], F32, name="warm", tag="proj")
                nc.tensor.matmul(pswu[:], wscr[:, 0:P], wscr[:],
                                 start=True, stop=True)

            # ---- input DMAs across THREE parallel DGE queues. Per-queue
            # transfers serialize and each dma_start costs ~2us fixed +
            # ~3us/MB, so the first load on each queue is what the critical
            # path sees: w1 leads the scalar queue, x0's first d-tiles lead
            # the sync queue, and the later-needed wv/biases ride the slower
            # gpsimd SWDGE queue. Every transfer is contiguous per
            # partition. ----
            HA = DHA * CH            # first-half fp16 elems per partition
            HB = (DT - DHA) * CH     # second-half elems
            w1_sb = const.tile([P, DT * P], F16, name="w1_sb")
            nc.scalar.dma_start(out=w1_sb[:], in_=w1d[:, :])
            b_sb = const.tile([P, 2], F32, name="b_sb")
            nc.gpsimd.dma_start(out=b_sb[:], in_=bb[:, :])
            wv_sb = const.tile([P, DT * HD], F16, name="wv_sb")
            nc.gpsimd.dma_start(out=wv_sb[:], in_=wvd[:, :])
            xq = {}
            for ci in (0, 1):
                ta = const.tile([P, HA], F16, name=f"xq{ci}a")
                tb = const.tile([P, HB], F16, name=f"xq{ci}b")
                base = ci * DT * CH
                nc.sync.dma_start(out=ta[:], in_=xp[:, base:base + HA])
                nc.scalar.dma_start(
                    out=tb[:], in_=xp[:, base + HA:base + DT * CH])
                xq[ci] = (ta, tb)
            for ci, eng in ((2, nc.sync), (3, nc.scalar)):
                t = const.tile([P, DT * CH], F16, name=f"xq{ci}")
                base = ci * DT * CH
                eng.dma_start(out=t[:], in_=xp[:, base:base + DT * CH])
                xq[ci] = (t,)

            def xqs(ci, d):
                parts = xq[ci]
                if len(parts) == 1:
                    return parts[0][:, d * CH:(d + 1) * CH]
                if d < DHA:
                    return parts[0][:, d * CH:(d + 1) * CH]
                return parts[1][:, (d - DHA) * CH:(d - DHA + 1) * CH]

            ident = const.tile([P, P], F16, name="ident")
            make_identity(nc, ident)

            def w1s(d):
                return w1_sb[:, d * P:(d + 1) * P]

            def wvs(d):
                return wv_sb[:, d * HD:(d + 1) * HD]

            # ---- activations ----
            # qk1: rows 0-63 = Q^T, rows 64-127 = K^T; qk2: swapped halves.
            qk1 = acts.tile([P, T], F16, name="qk1")
            qk2 = acts.tile([P, T], F16, name="qk2")
            vT = acts.tile([HD, T], F16, name="vT")
            v_aug = acts.tile([P, NKT, HD + 1], F16, name="v_aug")
            nc.vector.memset(v_aug[:, :, HD], 1.0)

            def qk_chunk(ci):
                cs = slice(ci * CH, (ci + 1) * CH)
                ps = proj_ps.tile([P, CH], F32, name="proj", tag="proj")
                for d in range(DT):
                    nc.tensor.matmul(ps[:], w1s(d), xqs(ci, d),
                                     start=(d == 0), stop=(d == DT - 1))
                nc.vector.tensor_scalar_add(qk1[:, cs], ps[:], b_sb[:, 0:1])
                # half-swapped copy: qk2 = [K^T; Q^T]. 64-partition DVE ops
                # read any aligned src half and write either dest half.
                nc.vector.tensor_copy(qk2[0:HD, cs], qk1[HD:P, cs])
                nc.vector.tensor_copy(qk2[HD:P, cs], qk1[0:HD, cs])

            def v_pair(ca, cb, cq=None):
                # V projections for two chunks col-packed: chunk ca on array
                # columns 0-63, chunk cb on columns 64-127 -> the matmul pairs
                # overlap in the PE array; outputs land in disjoint halves of
                # one PSUM bank. When cq is given, the QK projection of chunk
                # cq is interleaved matmul-by-matmul so each LDWEIGHTS hides
                # behind the other stream's matmul.
                psv = proj_ps.tile([P, CH], F32, name="projv", tag="proj")
                psq = (proj_ps.tile([P, CH], F32, name="proj", tag="proj")
                       if cq is not None else None)
                for d in range(DT):
                    nc.tensor.matmul(psv[0:HD, :], wvs(d), xqs(ca, d),
                                     start=(d == 0), stop=(d == DT - 1))
                    nc.tensor.matmul(psv[HD:P, :], wvs(d), xqs(cb, d),
                                     start=(d == 0), stop=(d == DT - 1))
                    if psq is not None:
                        nc.tensor.matmul(psq[:], w1s(d), xqs(cq, d),
                                         start=(d == 0), stop=(d == DT - 1))
                nc.vector.tensor_scalar_add(
                    vT[:, ca * CH:(ca + 1) * CH], psv[0:HD, :], b_sb[0:HD, 1:2])
                nc.vector.tensor_scalar_add(
                    vT[:, cb * CH:(cb + 1) * CH], psv[HD:P, :], b_sb[HD:P, 1:2])
                if psq is not None:
                    cs = slice(cq * CH, (cq + 1) * CH)
                    nc.vector.tensor_scalar_add(qk1[:, cs], psq[:],
                                                b_sb[:, 0:1])
                    nc.vector.tensor_copy(qk2[0:HD, cs], qk1[HD:P, cs])
                    nc.vector.tensor_copy(qk2[HD:P, cs], qk1[0:HD, cs])
                for tt in range(4 * ca, 4 * ca + 8):
                    tp = tr_ps.tile([P, HD], F16, name="vtr", tag="vtr")
                    nc.tensor.transpose(tp[:], vT[:, tt * P:(tt + 1) * P],
                                        ident[:HD, :HD])
                    nc.vector.tensor_copy(v_aug[:, tt, 0:HD], tp[:])

            def chunk_pairs(ci):
                # diagonal pairs first (col-trimmed, masked), then full pairs
                return ([(4 * ci, 4 * ci + 1), (4 * ci + 2, 4 * ci + 3)]
                        + [(2 * j, 2 * j + 1) for j in range(2 * ci)])

            def scores_pair(ci, ka, kb, diag):
                c0 = ci * CH
                da = max(ka * P - c0, 0)  # first unmasked column
                db = max(kb * P - c0, 0)
                s2 = s_ps.tile([P, 2 * CH], F32, name="sT", tag="sT")
                # rows 0-63 of the array: K^T from qk2, Q^T from qk1
                nc.tensor.matmul(s2[:, da:CH],
                                 qk2[0:HD, ka * P:(ka + 1) * P],
                                 qk1[0:HD, c0 + da:c0 + CH],
                                 start=True, stop=True)
                # rows 64-127: K^T from qk1, Q^T from qk2 (concurrent)
                nc.tensor.matmul(s2[:, CH + db:2 * CH],
                                 qk1[HD:P, kb * P:(kb + 1) * P],
                                 qk2[HD:P, c0 + db:c0 + CH],
                                 start=True, stop=True)
                pt = pwork.tile([P, 2 * CH], F16, name="pT", tag="pT")
                if diag:
                    nc.scalar.activation(pt[:, da:CH], s2[:, da:CH],
                                         mybir.ActivationFunctionType.Exp)
                    nc.scalar.activation(pt[:, CH + db:2 * CH],
                                         s2[:, CH + db:2 * CH],
                                         mybir.ActivationFunctionType.Exp)
                    # causal mask on the 128-col diagonal band only:
                    # keep where (query - delta) >= key  <=>  c' >= p
                    for off in (da, CH + db):
                        nc.gpsimd.affine_select(
                            out=pt[:, off:off + P],
                            in_=pt[:, off:off + P],
                            compare_op=mybir.AluOpType.is_ge, fill=0.0,
                            base=0, pattern=[[1, P]],
                            channel_multiplier=-1,
                        )
                else:
                    nc.scalar.activation(pt[:], s2[:],
                                         mybir.ActivationFunctionType.Exp)
                return pt

            def pv_pair(ci, ops, ka, kb, pt, first, last):
                c0 = ci * CH
                da = max(ka * P - c0, 0)
                db = max(kb * P - c0, 0)
                nc.tensor.matmul(ops[:, da:CH], v_aug[:, ka, :],
                                 pt[:, da:CH],
                                 start=first, stop=False)
                nc.tensor.matmul(ops[:, db:CH], v_aug[:, kb, :],
                                 pt[:, CH + db:2 * CH],
                                 start=False, stop=last)

            def store_chunk(ci, ops):
                osb = owork.tile([HD + 1, CH], F32, name="osb", tag="osb")
                nc.vector.tensor_copy(osb[:], ops[:])
                nc.sync.dma_start(
                    out=outT[:, ci * CH:(ci + 1) * CH], in_=osb[:])

            def attn_chunk(ci):
                # scores run one pair ahead of PV: the PV pair behind a
                # just-issued EXP never heads the PE queue, so its
                # LDWEIGHTS prefetches behind the next scores matmuls.
                ops = o_ps.tile([HD + 1, CH], F32, name="oacc", tag="oacc")
                pairs = chunk_pairs(ci)
                prev = None
                for idx, (ka, kb) in enumerate(pairs):
                    pt = scores_pair(ci, ka, kb, diag=idx < 2)
                    if prev is not None:
                        pv_pair(ci, ops, prev[0], prev[1], prev[2],
                                first=prev[3] == 0, last=False)
                    prev = (ka, kb, pt, idx)
                pv_pair(ci, ops, prev[0], prev[1], prev[2],
                        first=prev[3] == 0, last=True)
                store_chunk(ci, ops)

            qk_chunk(0)
            # chunk 0: scores+exp run before the V projection so the ACT
            # engine starts as early as possible; the V projection (with
            # chunk 1's QK projection interleaved) fills the PE while the
            # chunk-0 EXPs run; PV follows once v_aug is ready.
            p0 = chunk_pairs(0)
            pts = [scores_pair(0, ka, kb, diag=True) for ka, kb in p0]
            v_pair(0, 1, cq=1)
            ops0 = o_ps.tile([HD + 1, CH], F32, name="oacc", tag="oacc")
            for idx, (ka, kb) in enumerate(p0):
                pv_pair(0, ops0, ka, kb, pts[idx],
                        first=idx == 0, last=idx == len(p0) - 1)
            store_chunk(0, ops0)
            attn_chunk(1)
            v_pair(2, 3, cq=2)
            attn_chunk(2)
            qk_chunk(3)
            attn_chunk(3)

    if legalize:
        _legalize_waits(nc, mybir)
    return nc


def _legalize_waits(nc, mybir):
    """Split multi-wait instructions for the XLA-route walrus codegen.

    The TPB EVENTS struct holds one semaphore wait per instruction and this
    pipeline's codegen refuses >1. Hoist extra waits onto standalone
    EventSemaphore instructions on the same engine queue right before the
    instruction - semantically identical, the queue stalls there.
    """
    n = 0
    for f in nc.m.functions:
        for b in f.blocks:
            out = []
            changed = False
            for inst in b.instructions:
                si = inst.sync_info
                waits = list(si.on_wait) if si is not None and si.on_wait else []
                if len(waits) > 1:
                    changed = True
                    for w in waits[:-1]:
                        n += 1
                        out.append(mybir.InstEventSemaphore(
                            name=f"waitfix{n}_{inst.name}",
                            engine=inst.engine,
                            sync_info=mybir.SyncInfo(on_wait=[w], on_update=[]),
                        ))
                    inst.sync_info = mybir.SyncInfo(
                        on_wait=waits[-1:],
                        on_update=list(si.on_update or []),
                    )
                out.append(inst)
            if changed:
                b.instructions = out
    return n


def kernel(x, wq, bq, wk, bk, wv, bv):
    global LAST_RESULTS
    import os
    os.environ.setdefault("JAX_PLATFORMS", "")
    from concourse.bass_utils import run_bass_kernel_spmd

    x = np.asarray(x, dtype=np.float32)
    s = np.float32(1.0 / np.sqrt(HD))
    # per partition p (= row of the D-contraction tile), d-major columns
    w1 = np.concatenate([np.asarray(wq, np.float32) * s,
                         np.asarray(wk, np.float32)], 0).T  # (D, 128)
    w1d = np.ascontiguousarray(
        w1.reshape(DT, P, P).transpose(1, 0, 2)
        .reshape(P, DT * P)).astype(np.float16)
    wv_t = np.asarray(wv, np.float32).T                      # (D, 64)
    wvd = np.ascontiguousarray(
        wv_t.reshape(DT, P, HD).transpose(1, 0, 2)
        .reshape(P, DT * HD)).astype(np.float16)
    b1 = np.concatenate([np.asarray(bq, np.float32) * s,
                         np.asarray(bk, np.float32)])
    bv_f = np.asarray(bv, np.float32)
    bb = np.ascontiguousarray(
        np.stack([b1, np.concatenate([bv_f, bv_f])], axis=1))  # (P, 2)
    # xp[b]: partition-major, chunk-major, d-major: row p holds, for each
    # chunk ci and d-tile d, the 512 fp16 values x[b, ci*CH:(ci+1)*CH, d*P+p].
    xp = np.ascontiguousarray(
        x.reshape(B, NCH, CH, DT, P).transpose(0, 4, 1, 3, 2)
        .reshape(B, P, NCH * DT * CH)).astype(np.float16)

    nc = _build_module()
    in_maps = [
        {"xp": xp[b], "w1d": w1d, "wvd": wvd, "bb": bb}
        for b in range(B)
    ]
    res = None
    for attempt in range(3):
        try:
            res = run_bass_kernel_spmd(nc, in_maps, core_ids=list(range(B)))
            break
        except Exception:
            # transient device wedges (NRT_EXEC_UNIT_UNRECOVERABLE) happen;
            # rebuild the module and retry on a clean execution
            if attempt == 2:
                raise
            nc = _build_module()
    LAST_RESULTS = res

    out = np.empty((B, T, HD), dtype=np.float32)
    for b in range(B):
        oT = res.results[b]["outT"]  # (65, T): rows 0..63 = O^T, row 64 = denom
        out[b] = (oT[:HD] / oT[HD:HD + 1]).T
    return out


# revision 19
# speedup vs baseline: 1.0433x; 1.0433x over previous
"""Trainium2 Bass kernel for a single causal attention head.

Problem: x:(8,2048,1024) f32, per-head projections wq/wk/wv:(64,1024),
biases (64,). Output: softmax(causal(q k^T / sqrt(64))) @ v : (8,2048,64).

Strategy:
  - Data-parallel: batch b -> core b (8 cores, 1 batch each).
  - Host prep packs every input into partition-major, fully contiguous
    per-partition lines so each DMA is ~128 large descriptors:
      * xp:(P, NCH*DT*CH) fp16 - x[b] chunk-major/d-major per partition
        (8KB contiguous per partition per chunk).
      * wall:(P, DT*(P+HD)) fp16 - [wq*s|wk] and wv interleaved per d-tile.
      * bb:(P, 2) f32 - [bq*s;bk] and [bv;bv] columns.
  - Device (per core):
      * qk1 = [wq|wk]^T.T @ x: rows 0-63 = Q^T, rows 64-127 = K^T (PSUM
        accumulate over 8 d-tiles, fp16 matmuls, N=512 chunks).
      * qk2 = half-swapped copy of qk1 -> both Q^T and K^T live on both
        partition halves; scores for two k-tiles share the PE array via
        row packing.
      * vT (64,T) fp16, transposed back to (T,64) tiles via fp16 PE
        transpose, augmented with a ones column (softmax denominator
        rides along the PV matmul).
      * S^T = K^T.T @ Q^T per k-tile; P^T = exp(S^T) on ACT; causal mask
        via gpsimd affine_select restricted to the 128-col diagonal band.
      * Diagonal pairs run FIRST per chunk with column-trimmed scores/
        exp/mask/PV (fully-masked columns never computed); non-diagonal
        pairs follow full-range.
      * O^T_aug[65, T] accumulated in PSUM over k-tiles; row 64 = sum_j P^T.
      * attention for chunk ci emitted right after projection chunk ci.
  - Host post: out[b] = (O^T[0:64] / O^T[64:65]).T  (softmax normalization).
"""

import numpy as np

B, T, D, HD = 8, 2048, 1024, 64
P = 128          # SBUF partitions
CH = 512         # q-chunk (matmul moving dim)
NCH = T // CH    # 4
DT = D // P      # 8 d-tiles
NKT = T // P     # 16 k-tiles
NWARM = 11       # PE clock-ramp warmup matmuls
DH = DT // 2     # d-tiles per combined/half x load (two DMA queues)

LAST_RESULTS = None      # BassKernelResults of the most recent run (for test.py)


def _build_module(legalize=True):
    import concourse.bass as bass
    import concourse.mybir as mybir
    from concourse.tile import TileContext

    from concourse.masks import make_identity
    F32 = mybir.dt.float32
    F16 = mybir.dt.float16

    nc = bass.Bass("TRN2", target_bir_lowering=True)

    WXC = DH * (P + CH)  # cols of a combined [w1-half | x0-half] tensor
    xp = nc.dram_tensor("xp", (P, NCH * DT * CH), F16, kind="ExternalInput")
    wxa = nc.dram_tensor("wxa", (P, WXC), F16, kind="ExternalInput")
    wxb = nc.dram_tensor("wxb", (P, WXC), F16, kind="ExternalInput")
    wvd = nc.dram_tensor("wvd", (P, DT * HD), F16, kind="ExternalInput")
    bb = nc.dram_tensor("bb", (P, 2), F32, kind="ExternalInput")
    outT = nc.dram_tensor("outT", (HD + 1, T), F32, kind="ExternalOutput")

    with TileContext(nc) as tc:
        with (
            tc.tile_pool(name="const", bufs=1) as const,
            tc.tile_pool(name="acts", bufs=1) as acts,
            tc.tile_pool(name="proj_ps", bufs=2, space="PSUM") as proj_ps,
            tc.tile_pool(name="tr_ps", bufs=1, space="PSUM") as tr_ps,
            tc.tile_pool(name="s_ps", bufs=2, space="PSUM") as s_ps,
            tc.tile_pool(name="o_ps", bufs=1, space="PSUM") as o_ps,
            tc.tile_pool(name="pwork", bufs=6) as pwork,
            tc.tile_pool(name="owork", bufs=2) as owork,
        ):
            # ---- PE warm-up first: throwaway matmuls keep the PE busy
            # through its clock-ramp window so real matmuls run at full
            # speed. Gated only on the wscr memset, not on any DMA. Any PE
            # idle gap resets the clock ramp, so the warmup count is sized
            # to bridge until the first x half lands.
            wscr = const.tile([P, CH], F16, name="wscr")
            nc.vector.memset(wscr[:], 0.0)
            for wu in range(NWARM):
                pswu = proj_ps.tile([P, CH], F32, name="warm", tag="proj")
                nc.tensor.matmul(pswu[:], wscr[:, 0:P], wscr[:],
                                 start=True, stop=True)

            # ---- input DMAs across THREE parallel DGE queues. Per-queue
            # transfers serialize and each dma_start costs ~3.4us fixed on
            # the first load (~1us after) + ~3us/MB, so everything qk0 needs
            # rides the FIRST load of each queue: combined [w1-half |
            # x0-half] tensors on sync and scalar. The later-needed
            # wv/biases ride the slower gpsimd SWDGE queue. Every transfer
            # is contiguous per partition. ----
            HB = DH * CH             # x half-chunk fp16 elems per partition
            wx_a = const.tile([P, WXC], F16, name="wx_a")
            nc.sync.dma_start(out=wx_a[:], in_=wxa[:, :])
            wx_b = const.tile([P, WXC], F16, name="wx_b")
            nc.scalar.dma_start(out=wx_b[:], in_=wxb[:, :])
            b_sb = const.tile([P, 2], F32, name="b_sb")
            nc.gpsimd.dma_start(out=b_sb[:], in_=bb[:, :])
            wv_sb = const.tile([P, DT * HD], F16, name="wv_sb")
            nc.gpsimd.dma_start(out=wv_sb[:], in_=wvd[:, :])
            xq = {0: (wx_a, wx_b)}
            ta = const.tile([P, HB], F16, name="xq1a")
            tb = const.tile([P, HB], F16, name="xq1b")
            nc.sync.dma_start(out=ta[:], in_=xp[:, DT * CH:DT * CH + HB])
            nc.scalar.dma_start(
                out=tb[:], in_=xp[:, DT * CH + HB:2 * DT * CH])
            xq[1] = (ta, tb)
            for ci, eng in ((2, nc.sync), (3, nc.scalar)):
                t = const.tile([P, DT * CH], F16, name=f"xq{ci}")
                base = ci * DT * CH
                eng.dma_start(out=t[:], in_=xp[:, base:base + DT * CH])
                xq[ci] = (t,)

            def xqs(ci, d):
                parts = xq[ci]
                if len(parts) == 1:
                    return parts[0][:, d * CH:(d + 1) * CH]
                t = parts[0] if d < DH else parts[1]
                dd = d % DH
                off = DH * P if ci == 0 else 0
                return t[:, off + dd * CH:off + (dd + 1) * CH]

            ident = const.tile([P, P], F16, name="ident")
            make_identity(nc, ident)

            def w1s(d):
                t = wx_a if d < DH else wx_b
                dd = d % DH
                return t[:, dd * P:(dd + 1) * P]

            def wvs(d):
                return wv_sb[:, d * HD:(d + 1) * HD]

            # ---- activations ----
            # qk1: rows 0-63 = Q^T, rows 64-127 = K^T; qk2: swapped halves.
            qk1 = acts.tile([P, T], F16, name="qk1")
            qk2 = acts.tile([P, T], F16, name="qk2")
            vT = acts.tile([HD, T], F16, name="vT")
            v_aug = acts.tile([P, NKT, HD + 1], F16, name="v_aug")
            nc.vector.memset(v_aug[:, :, HD], 1.0)

            def qk_chunk(ci):
                cs = slice(ci * CH, (ci + 1) * CH)
                ps = proj_ps.tile([P, CH], F32, name="proj", tag="proj")
                for d in range(DT):
                    nc.tensor.matmul(ps[:], w1s(d), xqs(ci, d),
                                     start=(d == 0), stop=(d == DT - 1))
                nc.vector.tensor_scalar_add(qk1[:, cs], ps[:], b_sb[:, 0:1])
                # half-swapped copy: qk2 = [K^T; Q^T]. 64-partition DVE ops
                # read any aligned src half and write either dest half.
                nc.vector.tensor_copy(qk2[0:HD, cs], qk1[HD:P, cs])
                nc.vector.tensor_copy(qk2[HD:P, cs], qk1[0:HD, cs])

            def v_pair(ca, cb, cq=None):
                # V projections for two chunks col-packed: chunk ca on array
                # columns 0-63, chunk cb on columns 64-127 -> the matmul pairs
                # overlap in the PE array; outputs land in disjoint halves of
                # one PSUM bank. When cq is given, the QK projection of chunk
                # cq is interleaved matmul-by-matmul so each LDWEIGHTS hides
                # behind the other stream's matmul.
                psv = proj_ps.tile([P, CH], F32, name="projv", tag="proj")
                psq = (proj_ps.tile([P, CH], F32, name="proj", tag="proj")
                       if cq is not None else None)
                for d in range(DT):
                    nc.tensor.matmul(psv[0:HD, :], wvs(d), xqs(ca, d),
                                     start=(d == 0), stop=(d == DT - 1))
                    nc.tensor.matmul(psv[HD:P, :], wvs(d), xqs(cb, d),
                                     start=(d == 0), stop=(d == DT - 1))
                    if psq is not None:
                        nc.tensor.matmul(psq[:], w1s(d), xqs(cq, d),
                                         start=(d == 0), stop=(d == DT - 1))
                nc.vector.tensor_scalar_add(
                    vT[:, ca * CH:(ca + 1) * CH], psv[0:HD, :], b_sb[0:HD, 1:2])
                nc.vector.tensor_scalar_add(
                    vT[:, cb * CH:(cb + 1) * CH], psv[HD:P, :], b_sb[HD:P, 1:2])
                if psq is not None:
                    cs = slice(cq * CH, (cq + 1) * CH)
                    nc.vector.tensor_scalar_add(qk1[:, cs], psq[:],
                                                b_sb[:, 0:1])
                    nc.vector.tensor_copy(qk2[0:HD, cs], qk1[HD:P, cs])
                    nc.vector.tensor_copy(qk2[HD:P, cs], qk1[0:HD, cs])
                for tt in range(4 * ca, 4 * ca + 8):
                    tp = tr_ps.tile([P, HD], F16, name="vtr", tag="vtr")
                    nc.tensor.transpose(tp[:], vT[:, tt * P:(tt + 1) * P],
                                        ident[:HD, :HD])
                    nc.vector.tensor_copy(v_aug[:, tt, 0:HD], tp[:])

            def chunk_pairs(ci):
                # diagonal pairs first (col-trimmed, masked), then full pairs
                return ([(4 * ci, 4 * ci + 1), (4 * ci + 2, 4 * ci + 3)]
                        + [(2 * j, 2 * j + 1) for j in range(2 * ci)])

            def scores_pair(ci, ka, kb, diag):
                c0 = ci * CH
                da = max(ka * P - c0, 0)  # first unmasked column
                db = max(kb * P - c0, 0)
                s2 = s_ps.tile([P, 2 * CH], F32, name="sT", tag="sT")
                # rows 0-63 of the array: K^T from qk2, Q^T from qk1
                nc.tensor.matmul(s2[:, da:CH],
                                 qk2[0:HD, ka * P:(ka + 1) * P],
                                 qk1[0:HD, c0 + da:c0 + CH],
                                 start=True, stop=True)
                # rows 64-127: K^T from qk1, Q^T from qk2 (concurrent)
                nc.tensor.matmul(s2[:, CH + db:2 * CH],
                                 qk1[HD:P, kb * P:(kb + 1) * P],
                                 qk2[HD:P, c0 + db:c0 + CH],
                                 start=True, stop=True)
                pt = pwork.tile([P, 2 * CH], F16, name="pT", tag="pT")
                if diag:
                    nc.scalar.activation(pt[:, da:CH], s2[:, da:CH],
                                         mybir.ActivationFunctionType.Exp)
                    nc.scalar.activation(pt[:, CH + db:2 * CH],
                                         s2[:, CH + db:2 * CH],
                                         mybir.ActivationFunctionType.Exp)
                    # causal mask on the 128-col diagonal band only:
                    # keep where (query - delta) >= key  <=>  c' >= p
                    for off in (da, CH + db):
                        nc.gpsimd.affine_select(
                            out=pt[:, off:off + P],
                            in_=pt[:, off:off + P],
                            compare_op=mybir.AluOpType.is_ge, fill=0.0,
                            base=0, pattern=[[1, P]],
                            channel_multiplier=-1,
                        )
                else:
                    nc.scalar.activation(pt[:], s2[:],
                                         mybir.ActivationFunctionType.Exp)
                return pt

            def pv_pair(ci, ops, ka, kb, pt, first, last):
                c0 = ci * CH
                da = max(ka * P - c0, 0)
                db = max(kb * P - c0, 0)
                nc.tensor.matmul(ops[:, da:CH], v_aug[:, ka, :],
                                 pt[:, da:CH],
                                 start=first, stop=False)
                nc.tensor.matmul(ops[:, db:CH], v_aug[:, kb, :],
                                 pt[:, CH + db:2 * CH],
                                 start=False, stop=last)

            def store_chunk(ci, ops):
                osb = owork.tile([HD + 1, CH], F32, name="osb", tag="osb")
                nc.vector.tensor_copy(osb[:], ops[:])
                nc.sync.dma_start(
                    out=outT[:, ci * CH:(ci + 1) * CH], in_=osb[:])

            def attn_chunk(ci):
                # scores run one pair ahead of PV: the PV pair behind a
                # just-issued EXP never heads the PE queue, so its
                # LDWEIGHTS prefetches behind the next scores matmuls.
                ops = o_ps.tile([HD + 1, CH], F32, name="oacc", tag="oacc")
                pairs = chunk_pairs(ci)
                prev = None
                for idx, (ka, kb) in enumerate(pairs):
                    pt = scores_pair(ci, ka, kb, diag=idx < 2)
                    if prev is not None:
                        pv_pair(ci, ops, prev[0], prev[1], prev[2],
                                first=prev[3] == 0, last=False)
                    prev = (ka, kb, pt, idx)
                pv_pair(ci, ops, prev[0], prev[1], prev[2],
                        first=prev[3] == 0, last=True)
                store_chunk(ci, ops)

            qk_chunk(0)
            # chunk 0: scores+exp run before the V projection so the ACT
            # engine starts as early as possible; the V projection (with
            # chunk 1's QK projection interleaved) fills the PE while the
            # chunk-0 EXPs run; PV follows once v_aug is ready.
            p0 = chunk_pairs(0)
            pts = [scores_pair(0, ka, kb, diag=True) for ka, kb in p0]
            v_pair(0, 1, cq=1)
            ops0 = o_ps.tile([HD + 1, CH], F32, name="oacc", tag="oacc")
            for idx, (ka, kb) in enumerate(p0):
                pv_pair(0, ops0, ka, kb, pts[idx],
                        first=idx == 0, last=idx == len(p0) - 1)
            store_chunk(0, ops0)
            attn_chunk(1)
            v_pair(2, 3, cq=2)
            attn_chunk(2)
            qk_chunk(3)
            attn_chunk(3)

    if legalize:
        _legalize_waits(nc, mybir)
    return nc


def _legalize_waits(nc, mybir):
    """Split multi-wait instructions for the XLA-route walrus codegen.

    The TPB EVENTS struct holds one semaphore wait per instruction and this
    pipeline's codegen refuses >1. Hoist extra waits onto standalone
    EventSemaphore instructions on the same engine queue right before the
    instruction - semantically identical, the queue stalls there.
    """
    n = 0
    for f in nc.m.functions:
        for b in f.blocks:
            out = []
            changed = False
            for inst in b.instructions:
                si = inst.sync_info
                waits = list(si.on_wait) if si is not None and si.on_wait else []
                if len(waits) > 1:
                    changed = True
                    for w in waits[:-1]:
                        n += 1
                        out.append(mybir.InstEventSemaphore(
                            name=f"waitfix{n}_{inst.name}",
                            engine=inst.engine,
                            sync_info=mybir.SyncInfo(on_wait=[w], on_update=[]),
                        ))
                    inst.sync_info = mybir.SyncInfo(
                        on_wait=waits[-1:],
                        on_update=list(si.on_update or []),
                    )
                out.append(inst)
            if changed:
                b.instructions = out
    return n


def kernel(x, wq, bq, wk, bk, wv, bv):
    global LAST_RESULTS
    import os
    os.environ.setdefault("JAX_PLATFORMS", "")
    from concourse.bass_utils import run_bass_kernel_spmd

    x = np.asarray(x, dtype=np.float32)
    s = np.float32(1.0 / np.sqrt(HD))
    # per partition p (= row of the D-contraction tile), d-major columns
    w1 = np.concatenate([np.asarray(wq, np.float32) * s,
                         np.asarray(wk, np.float32)], 0).T  # (D, 128)
    w1d = np.ascontiguousarray(
        w1.reshape(DT, P, P).transpose(1, 0, 2)
        .reshape(P, DT * P)).astype(np.float16)
    wv_t = np.asarray(wv, np.float32).T                      # (D, 64)
    wvd = np.ascontiguousarray(
        wv_t.reshape(DT, P, HD).transpose(1, 0, 2)
        .reshape(P, DT * HD)).astype(np.float16)
    b1 = np.concatenate([np.asarray(bq, np.float32) * s,
                         np.asarray(bk, np.float32)])
    bv_f = np.asarray(bv, np.float32)
    bb = np.ascontiguousarray(
        np.stack([b1, np.concatenate([bv_f, bv_f])], axis=1))  # (P, 2)
    # xp[b]: partition-major, chunk-major, d-major: row p holds, for each
    # chunk ci and d-tile d, the 512 fp16 values x[b, ci*CH:(ci+1)*CH, d*P+p].
    xp = np.ascontiguousarray(
        x.reshape(B, NCH, CH, DT, P).transpose(0, 4, 1, 3, 2)
        .reshape(B, P, NCH * DT * CH)).astype(np.float16)
    # combined first loads: [w1 d-half | x0 d-half] per DMA queue
    DH = DT // 2
    wxa_b = [np.ascontiguousarray(np.concatenate(
        [w1d[:, :DH * P], xp[b, :, :DH * CH]], axis=1)) for b in range(B)]
    wxb_b = [np.ascontiguousarray(np.concatenate(
        [w1d[:, DH * P:], xp[b, :, DH * CH:DT * CH]], axis=1))
        for b in range(B)]

    nc = _build_module()
    in_maps = [
        {"xp": xp[b], "wxa": wxa_b[b], "wxb": wxb_b[b],
         "wvd": wvd, "bb": bb}
        for b in range(B)
    ]
    res = None
    for attempt in range(3):
        try:
            res = run_bass_kernel_spmd(nc, in_maps, core_ids=list(range(B)))
            break
        except Exception:
            # transient device wedges (NRT_EXEC_UNIT_UNRECOVERABLE) happen;
            # rebuild the module and retry on a clean execution
            if attempt == 2:
                raise
            nc = _build_module()
    LAST_RESULTS = res

    out = np.empty((B, T, HD), dtype=np.float32)
    for b in range(B):
        oT = res.results[b]["outT"]  # (65, T): rows 0..63 = O^T, row 64 = denom
        out[b] = (oT[:HD] / oT[HD:HD + 1]).T
    return out
